# revision 9
# baseline (speedup 1.0000x reference)
"""AFM sparse-attention kernel for 8 TRN2 NeuronCores.

Problem (per reference):
    value[b,i,j,:] = emb[b,i,:] * emb[b,j,:]                  [B,N,N,d]
    qk = LeakyReLU(value @ w_W.T + w_b, 0.01)
    logits = qk @ a_W (+ a_b, softmax-invariant)
    alphas = softmax(logits, axis=-1)[..., None]              [B,N,N,1]
    returns (alphas, value)

B=256, N=64, d=64.  Pure data parallel: batch sharded 32/core over 8 cores.

Per core, 16 groups of 2 batches:
  - value in DMA-native layout [i-partitions, (j,d)]: a K=4 bf16 selector
    matmul replicates hi/lo split-float emb rows across partitions into
    f32 PSUM (exact to ~1e-5), then one DVE multiply per chunk against a
    stride-0 view of the emb rows. Per-batch store = 1MB contiguous DMA.
  - qk[i,(j,e)] = emb[b].T @ WJ2, where WJ2[(b,d),(j,e)] = embT*wWT2 is
    built 2-batches-at-a-time in one [128,4096] DVE op (stride-0 views).
    w_b enters via a K=1 accumulate-matmul of a precomputed bias row.
  - LeakyReLU(y)@a_W = 0.01*(a@W)@value + 0.99*sum_e +-Relu(|a_e|y_e):
    linear term is an extra matmul; Relu on ACT (bf16 out); the sign-
    grouped e-sums are two tensor_reduce ops on the inner dim.
  - softmax over j on [128, 64] tiles (2 batches packed).
"""

import numpy as np

B, N, D = 256, 64, 64
NCORES = 8
BS = B // NCORES          # 32 batches per core
NG = BS // 2              # 16 two-batch groups
NEG_SLOPE = 0.01

_CACHE = {}


def _build_nc(pos_cnt):
    import concourse.bass as bass
    import concourse.bacc as bacc
    import concourse.mybir as mybir
    import concourse.tile as tile

    f32 = mybir.dt.float32
    bf16 = mybir.dt.bfloat16
    nc = bacc.Bacc()

    # embT2: per group, [128, 64]: rows 0-63 = emb[b0].T (d-major),
    #        rows 64-127 = emb[b1].T
    embT2 = nc.declare_dram_parameter("embT2", [NG, 128, N], bf16, isOutput=False)
    embF = nc.declare_dram_parameter("embF", [BS, N * D], f32, isOutput=False)
    embHL = nc.declare_dram_parameter("embHL", [NG, 4, N * D], bf16, isOutput=False)
    wWT2 = nc.declare_dram_parameter("wWT2", [128, N], f32, isOutput=False)
    wA = nc.declare_dram_parameter("wA", [128, 1], f32, isOutput=False)
    wbrow = nc.declare_dram_parameter("wbrow", [1, N * N], bf16, isOutput=False)
    ones2 = nc.declare_dram_parameter("ones2", [1, 128], bf16, isOutput=False)
    sel = nc.declare_dram_parameter("sel", [4, 128], bf16, isOutput=False)
    value_o = nc.declare_dram_parameter("value", [BS, N, N * D], f32, isOutput=True)
    alpha_o = nc.declare_dram_parameter("alphas", [BS * N, N], f32, isOutput=True)

    Pp = pos_cnt  # columns 0..Pp-1 positive a_W, Pp..63 negative

    with tile.TileContext(nc) as tc:
        with (
            tc.tile_pool(name="consts", bufs=1) as consts,
            tc.tile_pool(name="inp", bufs=3) as inp,
            tc.tile_pool(name="wj", bufs=2) as wjp,
            tc.tile_pool(name="big", bufs=2) as big,
            tc.tile_pool(name="lq", bufs=2) as lqp,
            tc.tile_pool(name="sm", bufs=3) as smp,
            tc.tile_pool(name="efp", bufs=2, space="PSUM") as efpp,
            tc.tile_pool(name="qkp", bufs=3, space="PSUM") as qkpp,
            tc.tile_pool(name="ap", bufs=2, space="PSUM") as app,
        ):
            WT = consts.tile([128, N], f32)
            nc.sync.dma_start(out=WT, in_=wWT2[:])
            WA = consts.tile([128, 1], f32)
            nc.sync.dma_start(out=WA, in_=wA[:])
            SEL = consts.tile([4, 128], bf16)
            nc.sync.dma_start(out=SEL, in_=sel[:])
            WB = consts.tile([1, N * N], bf16)
            nc.sync.dma_start(out=WB, in_=wbrow[:])
            ON2 = consts.tile([1, 128], bf16)
            nc.sync.dma_start(out=ON2, in_=ones2[:])

            wta = WT[:]
            # viewWT[(b,d), (j, e)] = WT[(b,d), e]   (j outer, stride 0)
            viewWT = bass.AP(tensor=wta.tensor, offset=wta.offset,
                             ap=[wta.ap[0], [0, N], [wta.ap[1][0], N]])

            for g in range(NG):
                b0, b1 = 2 * g, 2 * g + 1
                # ---- loads
                ET2 = inp.tile([128, N], bf16, tag="et2")
                nc.sync.dma_start(out=ET2, in_=embT2[g])
                EMB2 = inp.tile([4, N * D], bf16, tag="emb2")
                nc.sync.dma_start(out=EMB2, in_=embHL[g])
                E128 = inp.tile([128, D], f32, tag="e128")
                nc.sync.dma_start(
                    out=E128,
                    in_=embF[b0:b0 + 2].rearrange("b (i d) -> (b i) d", d=D))

                # ---- WJ2 build: one [128, N*N] DVE op for both batches
                WJ2 = wjp.tile([128, N * N], bf16, tag="wj2")
                eta = ET2[:]
                viewJ = bass.AP(tensor=eta.tensor, offset=eta.offset,
                                ap=[eta.ap[0], [eta.ap[1][0], N], [0, N]])
                nc.vector.tensor_mul(WJ2, viewJ, viewWT)

                # WJA[(b,d), j] = ET2 * WA  (A-term moving operand)
                WJA = inp.tile([128, N], bf16, tag="wja")
                nc.vector.tensor_scalar_mul(WJA, ET2, WA[:, 0:1])

                # ---- A-term matmuls -> PSUM [128, 64]
                Aps = app.tile([128, N], f32, tag="aps")
                nc.tensor.matmul(Aps[0:64, :], ET2[0:64, :], WJA[0:64, :],
                                 start=True, stop=True)
                nc.tensor.matmul(Aps[64:128, :], ET2[64:128, :], WJA[64:128, :],
                                 start=True, stop=True, tile_position=(64, 64))

                # ---- value + qk chunk loop (8 chunks of 512)
                V2 = big.tile([128, N * D], f32, tag="v2")
                LQ = lqp.tile([128, N * N], bf16, tag="lq")
                e1a = E128[:]
                for c in range(8):
                    sl = slice(512 * c, 512 * (c + 1))
                    # EF chunk: replicate hi/lo emb rows across halves
                    EF = efpp.tile([128, 512], f32, tag="ef")
                    nc.tensor.matmul(EF, SEL, EMB2[:, sl], start=True, stop=True)
                    # value chunk: E128 row (stride-0 over j) * EF
                    viewE = bass.AP(
                        tensor=e1a.tensor, offset=e1a.offset,
                        ap=[e1a.ap[0], [0, 8], [e1a.ap[1][0], D]])
                    nc.vector.tensor_mul(V2[:, sl], EF, viewE)
                    # qk chunk: both batches -> one [128, 512] psum tile
                    QK = qkpp.tile([128, 512], f32, tag="qk")
                    nc.tensor.matmul(QK[0:64, :], ET2[0:64, :], WJ2[0:64, sl],
                                     start=True, stop=True)
                    nc.tensor.matmul(QK[64:128, :], ET2[64:128, :],
                                     WJ2[64:128, sl],
                                     start=True, stop=True, tile_position=(64, 64))
                    # + w_b bias row (K=1 accumulate over all 128 partitions)
                    nc.tensor.matmul(QK, ON2, WB[:, sl], start=False, stop=True)
                    nc.scalar.activation(LQ[:, sl], QK,
                                         mybir.ActivationFunctionType.Relu)

                # ---- logits: sign-grouped reduces over e (inner dim)
                LG = smp.tile([128, N], f32, tag="lg")
                lqa = LQ[:]
                fs = lqa.ap[1][0]  # free step of LQ (elements)

                def lq_view(lo, cnt):
                    return bass.AP(tensor=lqa.tensor,
                                   offset=lqa.offset + fs * lo,
                                   ap=[lqa.ap[0], [fs * N, N], [fs, cnt]])

                if 0 < Pp < N:
                    RP = smp.tile([128, N], f32, tag="rp")
                    nc.vector.tensor_reduce(RP, lq_view(0, Pp),
                                            axis=mybir.AxisListType.X,
                                            op=mybir.AluOpType.add)
                    RN = smp.tile([128, N], f32, tag="rn")
                    nc.vector.tensor_reduce(RN, lq_view(Pp, N - Pp),
                                            axis=mybir.AxisListType.X,
                                            op=mybir.AluOpType.add, negate=True)
                    nc.vector.tensor_add(LG, RP, RN)
                else:
                    nc.vector.tensor_reduce(LG, lq_view(0, N),
                                            axis=mybir.AxisListType.X,
                                            op=mybir.AluOpType.add,
                                            negate=(Pp == 0))
                LG2 = smp.tile([128, N], f32, tag="lg2")
                nc.vector.tensor_add(LG2, LG, Aps)

                # ---- softmax over free dim (j)
                MX = smp.tile([128, 1], f32, tag="mx")
                nc.vector.tensor_reduce(MX, LG2, axis=mybir.AxisListType.X,
                                        op=mybir.AluOpType.max, negate=True)
                EX = smp.tile([128, N], f32, tag="ex")
                nc.scalar.activation(EX, LG2, mybir.ActivationFunctionType.Exp,
                                     bias=MX[:, 0:1], scale=1.0)
                SM = smp.tile([128, 1], f32, tag="sum")
                nc.vector.tensor_reduce(SM, EX, axis=mybir.AxisListType.X,
                                        op=mybir.AluOpType.add)
                RC = smp.tile([128, 1], f32, tag="rc")
                nc.vector.reciprocal(RC, SM)
                AL = smp.tile([128, N], f32, tag="al")
                nc.vector.tensor_scalar_mul(AL, EX, RC[:, 0:1])

                # ---- stores
                nc.sync.dma_start(out=alpha_o[128 * g:128 * (g + 1), :], in_=AL)
                nc.sync.dma_start(out=value_o[b0], in_=V2[0:64, :])
                nc.sync.dma_start(out=value_o[b1], in_=V2[64:128, :])
    nc.finalize()
    return nc


def _prep_host(inputs):
    import ml_dtypes
    bf = ml_dtypes.bfloat16

    emb = np.asarray(inputs["embeddings"], np.float32)       # [B, N, D]
    w_W = np.asarray(inputs["w_W"], np.float32)              # [e, d]
    w_b = np.asarray(inputs["w_b"], np.float32)              # [e]
    a_W = np.asarray(inputs["a_W"], np.float32)              # [e]

    pos = np.where(a_W >= 0)[0]
    neg = np.where(a_W < 0)[0]
    perm = np.concatenate([pos, neg])
    absa = np.abs(a_W[perm]) * (1.0 - NEG_SLOPE)

    wWT2 = np.empty((128, N), np.float32)
    wWT2[:D, :] = w_W[perm].T * absa[None, :]
    wWT2[D:, :] = wWT2[:D, :]

    wb2 = (w_b[perm] * absa).astype(np.float32)
    wbrow = np.tile(wb2, N)[None, :].astype(bf)              # [1, N*N] (j,e)

    wA = np.empty((128, 1), np.float32)
    wA[:D, 0] = NEG_SLOPE * (w_W.T @ a_W)
    wA[D:, 0] = wA[:D, 0]

    ones2 = np.ones((1, 128), bf)

    sel = np.zeros((4, 128), bf)
    sel[0, :64] = 1.0
    sel[1, :64] = 1.0
    sel[2, 64:] = 1.0
    sel[3, 64:] = 1.0

    embT2 = np.ascontiguousarray(
        emb.reshape(B // 2, 2, N, D).transpose(0, 1, 3, 2).reshape(B // 2, 128, N)
    ).astype(bf)

    embF = emb.reshape(B, N * D)
    hi = embF.astype(bf)
    lo = (embF - hi.astype(np.float32)).astype(bf)
    embHL = np.empty((B // 2, 4, N * D), bf)
    embHL[:, 0] = hi[0::2]
    embHL[:, 1] = lo[0::2]
    embHL[:, 2] = hi[1::2]
    embHL[:, 3] = lo[1::2]
    return embT2, embF, embHL, wWT2, wA, wbrow, ones2, sel, len(pos)


def _get_nc_and_maps(inputs):
    embT2, embF, embHL, wWT2, wA, wbrow, ones2, sel, pos_cnt = _prep_host(inputs)
    key = ("nc", pos_cnt)
    if key not in _CACHE:
        _CACHE[key] = _build_nc(pos_cnt)
    nc = _CACHE[key]
    in_maps = []
    for c in range(NCORES):
        s = slice(c * BS, (c + 1) * BS)
        in_maps.append({
            "embT2": embT2[c * NG:(c + 1) * NG], "embF": embF[s],
            "embHL": embHL[c * NG:(c + 1) * NG],
            "wWT2": wWT2, "wA": wA, "wbrow": wbrow, "ones2": ones2, "sel": sel,
        })
    return nc, in_maps


def kernel(**inputs):
    from concourse.bass_utils import run_bass_kernel_spmd

    nc, in_maps = _get_nc_and_maps(inputs)
    res = run_bass_kernel_spmd(nc, in_maps, core_ids=list(range(NCORES)))

    value = np.empty((B, N, N, D), np.float32)
    alphas = np.empty((B, N, N, 1), np.float32)
    for c in range(NCORES):
        r = res.results[c]
        value[c * BS:(c + 1) * BS] = r["value"].reshape(BS, N, N, D)
        alphas[c * BS:(c + 1) * BS] = r["alphas"].reshape(BS, N, N, 1)
    return alphas, value
